# revision 33
# baseline (speedup 1.0000x reference)
"""GPT-J joint attention (B=1, S=2048, D=2048, H=16, HD=128) on 8 Trainium2
NeuronCores, tensor-parallel over heads (2 heads per core).

Per-core program (all matmuls bf16 inputs, fp32 PSUM accumulation):
  Phase 1: QT/KT = W[qk]_shard @ hidden^T ([hd, s] layout, per head), with
    RoPE applied via a rotation-matrix matmul (the pair swap crosses the
    partition dim) + elementwise combine.
  Phase 2: one window per q-block. The exp-heavy score loop is interleaved
    per-kt-step with pure-PE filler (V projection groups of this window and
    out-projection blocks of the previous one) so the PE stays the pacing
    engine instead of the scalar engine's exp stream:
    - scores^T tiles = KT_tile^T . QT_block ([k, q] layout) -> exp -> causal
      mask on the [128,128] diagonal strip; diagonal tiles computed only over
      the causally-needed q range.
    - O^T accumulated as V_tile^T . P^T; softmax denominator via ones-matmul
      over 2 bf16 partial-sum lanes; 1/den via Ln+Exp on the scalar engine.
    - out_proj partials stream to DRAM per row-block.

Host side: shard/transpose/cast inputs into SBUF-image layouts (contiguous
per-partition lines => large DMA descriptors), run SPMD on 8 cores, sum the
8 partial outputs (the tensor-parallel all-reduce equivalent).
"""
import sys

import numpy as np
import ml_dtypes

try:
    import concourse.bass as bass
except ImportError:  # pragma: no cover
    sys.path.insert(0, "/opt/trn_rl_repo")
    import concourse.bass as bass

import concourse.mybir as mybir
import concourse.tile as tile
from concourse.bass_utils import run_bass_kernel_spmd

BF16 = mybir.dt.bfloat16
F32 = mybir.dt.float32
NPBF16 = ml_dtypes.bfloat16

N_CORES = 8
S = 2048          # sequence length
D = 2048          # model dim
HD = 128          # head dim
NHC = 2           # heads per core
DC = NHC * HD     # shard width (256)
P = 128           # partitions
KD = D // P       # 16 contraction tiles over model dim
QBS = 512         # q-block size
NQB = S // QBS    # 4 q-blocks
NST = S // P      # 16 sequence tiles of 128
SCALE = 1.0 / float(np.sqrt(HD))

# per-window schedules: which kt step hosts which V-projection group /
# out-projection block (pure PE filler spread through the exp-heavy loop)
V_STEPS = {
    0: {0: 0, 1: 1, 2: 2, 3: 3},
    1: {0: 0, 2: 1, 4: 2, 6: 3},
    2: {0: 0, 3: 1, 6: 2, 9: 3},
    3: {0: 0, 2: 1, 6: 2, 10: 3},
}
OP_STEPS = {
    1: {3: 0, 5: 1, 7: 2},
    2: {4: 0, 7: 1, 10: 2},
    3: {4: 0, 8: 1, 12: 2},
}

# ---------------------------------------------------------------------------
# Walrus's CoreV3 drain encoding accepts a single sem wait; Tile's tail drain
# carries one wait per logical proc. Split it into one drain per proc.
# ---------------------------------------------------------------------------


def _install_drain_split():
    if getattr(tile.TileContext, "_drain_split_installed", False):
        return
    from concourse.vector_clock import ScopedClock, VectorClock

    def _drain_and_barrier(self, tick_clock, wait_clock):
        full = tick_clock.global_clock
        n = len(full)
        for i in range(n):
            if full[i] <= 0:
                continue
            vec = [full[j] if j == i else 0 for j in range(n)]
            drain_inst = self.nc.sync.drain()
            wait_clock.add_sem_waits(
                drain_inst.ins, ScopedClock({None: VectorClock(vec)})
            )
        self.nc.all_engine_barrier()
        assert self.sems is not None
        popped = self.nc._tile_sem_poison_stack.pop()
        assert popped is self._sem_poison
        self.nc.clear_and_free_semaphores(list(self.sems.allocated().values()))
        self.nc.all_engine_barrier()

    tile.TileContext._drain_and_barrier = _drain_and_barrier
    tile.TileContext._drain_split_installed = True


def _split_excess_waits(nc, limit=1):
    """This walrus build rejects instructions carrying more than one sem wait
    (CoreV3 setupSyncWait: 'Too many sync wait commands'). Spill excess waits
    onto same-engine NOPs inserted just before the instruction — the engine
    executes them in queue order, so blocking semantics are unchanged."""
    ctr = 0
    for fn in nc.m.functions:
        for blk in fn.blocks:
            new_list = []
            for inst in blk.instructions:
                si = inst.sync_info
                if si is not None and len(si.on_wait) > limit:
                    waits = list(si.on_wait)
                    excess, keep = waits[:-limit], waits[-limit:]
                    for w in excess:
                        ctr += 1
                        nop = mybir.InstNoOp(
                            name=f"I-wsplit-{ctr}", text_hint="wait_split"
                        )
                        nop.engine = inst.engine
                        nop.sync_info = mybir.SyncInfo(on_wait=[w], on_update=[])
                        new_list.append(nop)
                    inst.sync_info = mybir.SyncInfo(
                        on_wait=keep, on_update=si.on_update
                    )
                new_list.append(inst)
            if len(new_list) != len(blk.instructions):
                blk.instructions[:] = new_list
    return ctr


def build_nc(split_waits=True):
    _install_drain_split()
    nc = bass.Bass()

    # All inputs are pre-laid-out on the host as SBUF images (partition dim
    # first, contiguous free dim) so each DMA moves large contiguous lines.
    hT = nc.dram_tensor("hT", [P, NQB, KD, QBS], BF16, kind="ExternalInput")
    wq = nc.dram_tensor("wq", [P, KD, DC], BF16, kind="ExternalInput")
    wk = nc.dram_tensor("wk", [P, KD, DC], BF16, kind="ExternalInput")
    wv = nc.dram_tensor("wv", [P, KD, DC], BF16, kind="ExternalInput")
    wo = nc.dram_tensor("wo", [P, NHC, D], BF16, kind="ExternalInput")
    ct = nc.dram_tensor("ct", [P, S], BF16, kind="ExternalInput")
    st = nc.dram_tensor("st", [P, S], BF16, kind="ExternalInput")
    rot = nc.dram_tensor("rot", [P, P], BF16, kind="ExternalInput")
    out = nc.dram_tensor("out", [S, D], BF16, kind="ExternalOutput")

    Exp = mybir.ActivationFunctionType.Exp
    Copy = mybir.ActivationFunctionType.Copy
    Ln = mybir.ActivationFunctionType.Ln

    with tile.TileContext(nc) as tc:
        with (
            tc.tile_pool(name="const", bufs=1) as const,
            tc.tile_pool(name="acts", bufs=1) as acts,
            tc.tile_pool(name="work", bufs=2) as work,
            tc.tile_pool(name="ptpool", bufs=6) as ptpool,
            tc.tile_pool(name="outstage", bufs=3) as outstage,
            tc.tile_pool(name="ps_sc", bufs=3, space="PSUM") as ps_sc,
            tc.tile_pool(name="ps_mm", bufs=2, space="PSUM") as ps_mm,
            tc.tile_pool(name="ps_ot", bufs=2, space="PSUM") as ps_ot,
            tc.tile_pool(name="ps_misc", bufs=1, space="PSUM") as ps_misc,
        ):
            wq_sb = const.tile([P, KD, DC], BF16)
            wk_sb = const.tile([P, KD, DC], BF16)
            wv_sb = const.tile([P, KD, DC], BF16)
            hT_sb = const.tile([P, NQB, KD, QBS], BF16)
            ct_sb = const.tile([P, S], BF16)
            st_sb = const.tile([P, S], BF16)
            rot_sb = const.tile([P, P], BF16)
            wo_sb = const.tile([P, NHC, D], BF16)

            # Single HWDGE trigger queue (sync), strictly ordered by first
            # use. Parallel queues were tried and regressed: they share the
            # 8 DMA-completion sem lanes + SDMA bandwidth, so secondary loads
            # ended up blocking the startup-critical hidden-state chunks.
            # first two kd-groups split in half so the very first matmuls
            # start ~2us earlier on the cold DMA pipe
            groups = [(0, 2), (2, 4), (4, 8), (8, 12), (12, 16)]
            for lo, hi in groups:
                gs = slice(lo, hi)
                nc.sync.dma_start(out=wq_sb[:, gs, :], in_=wq[:, gs, :])
                nc.sync.dma_start(
                    out=hT_sb[:, 0, gs, :], in_=hT[:, 0, gs, :]
                )
            nc.sync.dma_start(out=wk_sb[:, 0:8, :], in_=wk[:, 0:8, :])
            nc.sync.dma_start(out=wk_sb[:, 8:16, :], in_=wk[:, 8:16, :])
            nc.sync.dma_start(out=rot_sb, in_=rot[:, :])
            nc.sync.dma_start(out=ct_sb, in_=ct[:, :])
            nc.sync.dma_start(out=st_sb, in_=st[:, :])
            for qb in range(1, NQB):
                nc.sync.dma_start(
                    out=hT_sb[:, qb, :, :], in_=hT[:, qb, :, :]
                )
            nc.sync.dma_start(out=wv_sb, in_=wv[:, :, :])
            nc.sync.dma_start(out=wo_sb, in_=wo[:, :, :])

            ones_colb = const.tile([P, 1], BF16)   # lhsT for k-partition sums
            nc.vector.memset(ones_colb, 1.0)
            ones_row = const.tile([1, P], BF16)    # lhsT for partition broadcast
            nc.vector.memset(ones_row, 1.0)

            # persistent activations
            qt_sb = acts.tile([P, NHC, S], BF16)   # [hd, h, s] rotary-applied Q^T
            kt_sb = acts.tile([P, NHC, S], BF16)
            v_sb = acts.tile([P, NST, DC], BF16)   # [s%128, s//128, head*hd]
            otb_sb = acts.tile([P, NHC, S], BF16)  # normalized O^T per head

            # ---- phase 1: Q/K projections + rope, q-block at a time ----
            # Both heads' accumulation groups interleave per kd-chunk so each
            # arriving DMA chunk immediately feeds 8 matmuls (the cold DMA
            # pipe can't sustain a one-head burst). The rope epilogue is two
            # stages: the psum->sbuf copy is emitted at the next q-block's
            # start (frees the PSUM banks for reuse), while the rotation
            # matmul + 3 DVE ops slot between later matmul groups so the PE
            # never waits on them.
            pend_raw = []  # (psum, dst_sb, h, qb) awaiting ACT copy
            pend_fin = []  # (raw, dst_sb, h, qb) awaiting rot matmul + DVE
            raw_ctr = [0]

            def emit_raws():
                while pend_raw:
                    ps, dst_sb, h, qb = pend_raw.pop(0)
                    raw = work.tile(
                        [P, QBS], BF16, tag=f"raw{raw_ctr[0] % 4}"
                    )
                    raw_ctr[0] += 1
                    nc.scalar.activation(raw, ps, Copy)
                    pend_fin.append((raw, dst_sb, h, qb))

            def finish_one():
                if not pend_fin:
                    return
                raw, dst_sb, h, qb = pend_fin.pop(0)
                sl = slice(qb * QBS, (qb + 1) * QBS)
                rps = ps_mm.tile([P, QBS], F32, tag="mm")
                nc.tensor.matmul(rps, lhsT=rot_sb, rhs=raw, start=True, stop=True)
                t1 = work.tile([P, QBS], BF16, tag="t1")
                t2 = work.tile([P, QBS], BF16, tag="t2")
                nc.vector.tensor_mul(t1, raw, ct_sb[:, sl])
                nc.vector.tensor_mul(t2, rps, st_sb[:, sl])
                nc.vector.tensor_add(dst_sb[:, h, sl], t1, t2)

            def qk_pass(w_sb, dst_sb, qb, pool1):
                ps0 = ps_sc.tile([P, QBS], F32, tag="sc")
                ps1 = pool1.tile(
                    [P, QBS], F32, tag="sc" if pool1 is ps_sc else "mm"
                )
                for g in range(4):
                    if g in (1, 3):
                        finish_one()
                    for h, ps in ((0, ps0), (1, ps1)):
                        for kd in range(g * 4, (g + 1) * 4):
                            nc.tensor.matmul(
                                ps,
                                lhsT=w_sb[:, kd, h * HD:(h + 1) * HD],
                                rhs=hT_sb[:, qb, kd, :],
                                start=(kd == 0),
                                stop=(kd == KD - 1),
                                skip_group_check=True,
                            )
                pend_raw.append((ps0, dst_sb, 0, qb))
                pend_raw.append((ps1, dst_sb, 1, qb))

            for qb in range(NQB):
                emit_raws()
                qk_pass(wq_sb, qt_sb, qb, ps_sc)
                qk_pass(wk_sb, kt_sb, qb, ps_mm)
            emit_raws()
            while pend_fin:
                finish_one()

            # ---- phase 2: per q-block window with interleaved PE filler ----
            def v_group(qb, s4):
                st_idx = qb * 4 + s4
                ps = ps_mm.tile([P, DC], F32, tag="mm")
                for kd in range(KD):
                    nc.tensor.matmul(
                        ps,
                        lhsT=hT_sb[:, qb, kd, s4 * P:(s4 + 1) * P],
                        rhs=wv_sb[:, kd, :],
                        start=(kd == 0),
                        stop=(kd == KD - 1),
                    )
                nc.scalar.activation(v_sb[:, st_idx, :], ps, Copy)

            def op_block(src_qb, s4, chunked=0):
                # chunked: 0 = one DMA per row-block, 1 = halves. Both h0
                # matmuls of an eb-pair are emitted before the h1 matmuls so
                # the block only waits on head-0's normalization to start.
                st_idx = src_qb * 4 + s4
                ost = outstage.tile([P, D], BF16, tag="ost")
                for ebp in (0, 2):
                    pss = [
                        ps_mm.tile([P, QBS], F32, name="ops", tag="mm")
                        for _ in range(2)
                    ]
                    for h in range(NHC):
                        for i in range(2):
                            eb = ebp + i
                            nc.tensor.matmul(
                                pss[i],
                                lhsT=otb_sb[:, h, st_idx * P:(st_idx + 1) * P],
                                rhs=wo_sb[:, h, eb * QBS:(eb + 1) * QBS],
                                start=(h == 0),
                                stop=(h == NHC - 1),
                                skip_group_check=True,
                            )
                    for i in range(2):
                        eb = ebp + i
                        osl = ost[:, eb * QBS:(eb + 1) * QBS]
                        if i == 0:
                            nc.scalar.activation(osl, pss[i], Copy)
                        else:
                            nc.vector.tensor_copy(osl, pss[i])
                    if chunked:
                        hsl = slice(ebp * QBS, (ebp + 2) * QBS)
                        nc.sync.dma_start(
                            out=out[st_idx * P:(st_idx + 1) * P, hsl],
                            in_=ost[:, hsl],
                        )
                if not chunked:
                    nc.sync.dma_start(
                        out=out[st_idx * P:(st_idx + 1) * P, :], in_=ost
                    )

            # deferred normalization of the PREVIOUS window: broadcast 1/den
            # over partitions (ones-matmul) and scale O^T. Emitted early in
            # the next window so the ACT Ln/Exp latency hides under V work.
            pending_norm = []  # (qsl, den_sbs, ot_pss)

            def flush_norm():
                while pending_norm:
                    qsl, den_sbs, ot_pss = pending_norm.pop(0)
                    for h in range(NHC):
                        bc_ps = ps_misc.tile(
                            [P, QBS], F32, name="bc_ps", tag="ps_dm"
                        )
                        nc.tensor.matmul(
                            bc_ps, lhsT=ones_row, rhs=den_sbs[h],
                            start=True, stop=True,
                        )
                        bc_sb = work.tile([P, QBS], F32, tag=f"bc{h}")
                        nc.scalar.activation(bc_sb, bc_ps, Copy)
                        nc.vector.tensor_mul(otb_sb[:, h, qsl], ot_pss[h], bc_sb)

            OT_LAG = 3  # P.V matmul trails the score matmul so its sem wait
            # is already satisfied and LDWEIGHTS pipelines.
            for qb in range(NQB):
                qsl = slice(qb * QBS, (qb + 1) * QBS)
                kmax = (qb + 1) * 4
                v_steps = V_STEPS[qb]
                op_steps = OP_STEPS.get(qb, {})

                pts = {}
                offs = {}
                ot_pss = [
                    ps_ot.tile([P, QBS], F32, name="ot_ps", tag="ps_ot")
                    for _ in range(NHC)
                ]
                # 2-lane bf16 partial sums of exp tiles (softmax denominator).
                # Each lane tracks the q-offset below which it holds no data
                # (diagonal tiles are only computed on their causal q range).
                accs = [[None] * 2 for _ in range(NHC)]

                def acc_pt(h, kt, pt, off):
                    lane = kt % 2
                    if accs[h][lane] is None:
                        acc = work.tile(
                            [P, QBS], BF16, name=f"za{h}_{lane}",
                            tag=f"za{h}_{lane}",
                        )
                        nc.vector.tensor_copy(acc[:, off:], pt[:, off:])
                        accs[h][lane] = (acc, off)
                    else:
                        acc, aoff = accs[h][lane]
                        assert aoff <= off
                        nc.vector.tensor_add(
                            acc[:, off:], acc[:, off:], pt[:, off:]
                        )

                def pv_step(kt):
                    off = offs[kt]
                    for h in range(NHC):
                        nc.tensor.matmul(
                            ot_pss[h][:, off:],
                            lhsT=v_sb[:, kt, h * HD:(h + 1) * HD],
                            rhs=pts[(h, kt)][:, off:],
                            start=(kt == 0),
                            stop=(kt == kmax - 1),
                            skip_group_check=True,
                        )

                for kt in range(kmax):
                    if kt == 1:
                        flush_norm()
                    if kt in v_steps:
                        v_group(qb, v_steps[kt])
                    if kt in op_steps:
                        op_block(qb - 1, op_steps[kt])
                    j = kt - qb * 4
                    off = max(j, 0) * P  # causal: q < kt*128 never needed
                    offs[kt] = off
                    for h in range(NHC):
                        sps = ps_sc.tile([P, QBS], F32, tag="sc")
                        nc.tensor.matmul(
                            sps[:, off:],
                            lhsT=kt_sb[:, h, kt * P:(kt + 1) * P],
                            rhs=qt_sb[:, h, qb * QBS + off:(qb + 1) * QBS],
                            start=True,
                            stop=True,
                        )
                        pt = ptpool.tile([P, QBS], BF16, tag=f"pt{h}")
                        nc.scalar.activation(
                            pt[:, off:], sps[:, off:], Exp, scale=SCALE
                        )
                        if j >= 0:  # diagonal strip: causal mask (on Pool)
                            nc.gpsimd.affine_select(
                                out=pt[:, off:off + P],
                                in_=pt[:, off:off + P],
                                compare_op=mybir.AluOpType.is_ge,
                                fill=0.0,
                                base=0,
                                pattern=[[1, P]],
                                channel_multiplier=-1,
                            )
                        pts[(h, kt)] = pt
                        acc_pt(h, kt, pt, off)
                    if kt >= OT_LAG:
                        pv_step(kt - OT_LAG)
                for kt2 in range(max(kmax - OT_LAG, 0), kmax):
                    pv_step(kt2)

                if qb > 0:
                    # last out-proj block of the previous window: emitted
                    # before the den matmuls so the PE is busy while the DVE
                    # finishes the denominator lane adds and ACT runs Ln/Exp
                    op_block(qb - 1, 3)
                # softmax denominator; 1/den as exp(-ln(den)) on ACT (the DVE
                # iterative reciprocal takes ~3.3us and blocks the DVE queue)
                den_sbs = []
                for h in range(NHC):
                    den_ps = ps_misc.tile([1, QBS], F32, name="den_ps", tag="ps_dm")
                    lanes = [a for a in accs[h] if a is not None]
                    for li, (lane, aoff) in enumerate(lanes):
                        nc.tensor.matmul(
                            den_ps[:, aoff:], lhsT=ones_colb, rhs=lane[:, aoff:],
                            start=(li == 0), stop=(li == len(lanes) - 1),
                            skip_group_check=True,
                        )
                    ln_sb = work.tile([1, QBS], F32, tag=f"ln{h}")
                    nc.scalar.activation(ln_sb, den_ps, Ln)
                    r_bf = work.tile([1, QBS], BF16, tag=f"rb{h}")
                    nc.scalar.activation(r_bf, ln_sb, Exp, scale=-1.0)
                    den_sbs.append(r_bf)
                pending_norm.append((qsl, den_sbs, ot_pss))

            # tail: normalize the last window and stream its out-projection
            flush_norm()
            for s4 in range(4):
                op_block(NQB - 1, s4, chunked=1)
    if split_waits:
        _split_excess_waits(nc)
    return nc


_NC_CACHE = {}


def _get_nc():
    if "nc" not in _NC_CACHE:
        _NC_CACHE["nc"] = build_nc()
    return _NC_CACHE["nc"]


def _rotation_matrix_T():
    # rot(x)[2i] = -x[2i+1]; rot(x)[2i+1] = x[2i].  R[i,j] coefficient of x[j].
    R = np.zeros((HD, HD), np.float32)
    idx = np.arange(0, HD, 2)
    R[idx, idx + 1] = -1.0
    R[idx + 1, idx] = 1.0
    return np.ascontiguousarray(R.T)


def prepare_in_maps(hidden_states, sin, cos, Wq, Wk, Wv, Wo):
    hidden_states = np.asarray(hidden_states, dtype=np.float32)
    sin = np.asarray(sin, dtype=np.float32)
    cos = np.asarray(cos, dtype=np.float32)
    Wq = np.asarray(Wq, dtype=np.float32)
    Wk = np.asarray(Wk, dtype=np.float32)
    Wv = np.asarray(Wv, dtype=np.float32)
    Wo = np.asarray(Wo, dtype=np.float32)

    hs = hidden_states[0]  # [s, d]
    # hT image [p, qb, kd, sq]: element = hidden[qb*512+sq, kd*128+p]
    hT = np.ascontiguousarray(
        hs.reshape(NQB, QBS, KD, P).transpose(3, 0, 2, 1)
    ).astype(NPBF16)
    ct = np.ascontiguousarray(np.repeat(cos, 2, axis=1).T).astype(NPBF16)
    st = np.ascontiguousarray(np.repeat(sin, 2, axis=1).T).astype(NPBF16)
    rot = _rotation_matrix_T().astype(NPBF16)

    def w_img(W, c):  # [p, kd, e] with e local to the shard
        e0 = c * DC
        # element = W[e0+e, kd*128+p]
        return np.ascontiguousarray(
            W[e0:e0 + DC, :].T.reshape(KD, P, DC).transpose(1, 0, 2)
        ).astype(NPBF16)

    in_maps = []
    for c in range(N_CORES):
        e0 = c * DC
        wo_img = np.ascontiguousarray(
            Wo[:, e0:e0 + DC].T.reshape(NHC, P, D).transpose(1, 0, 2)
        ).astype(NPBF16)
        in_maps.append(
            {
                "hT": hT,
                "wq": w_img(Wq, c),
                "wk": w_img(Wk, c),
                "wv": w_img(Wv, c),
                "wo": wo_img,
                "ct": ct,
                "st": st,
                "rot": rot,
            }
        )
    return in_maps


def kernel(hidden_states, attention_mask, sin, cos, Wq, Wk, Wv, Wo):
    in_maps = prepare_in_maps(hidden_states, sin, cos, Wq, Wk, Wv, Wo)
    nc = _get_nc()
    res = run_bass_kernel_spmd(nc, in_maps, list(range(N_CORES)))
    out = res.results[0]["out"].astype(np.float32)
    for c in range(1, N_CORES):
        out += res.results[c]["out"].astype(np.float32)
    return out[None]
